# revision 2
# baseline (speedup 1.0000x reference)
"""Trainium2 Bass kernel for nn_EventSequenceEmbedder.

Strategy
--------
The whole module is algebraically folded on the host into a single small
matrix product per token:

    out[t, :] = featT[:, t] . M  (masked)

where
  * M [104, 256] is built once from the weights: each embedding table and
    each linear projection is folded through its combine_W column block
    (pure weight preprocessing), all biases collapse into one bias row.
  * featT [104, BS] is the per-token sparse feature vector (row order):
      rows 0:53    card multihot (counts of the 7 card ids; /7 folded in M)
      rows 53:62   hero one-hot
      rows 62:71   acting one-hot
      rows 71:81   num_players one-hot
      row  81      ones (bias row)
      rows 82:103  raw numeric features (scalars2, blinds2, bets9, action8)
      row  103     zero padding
    The whole featT is scaled by mask, which reproduces `out * mask` exactly.

Sharding: data-parallel over tokens, 8 contiguous blocks of 4096.

Per-core device program (v2 — DMA-roofline oriented):
  * Input: rows 0:82 (one-hot counts + bias: small integers, EXACTLY
    representable in fp8e4) are stored in DRAM as fp8 and cast to fp16
    during the SWDGE DMA (gpsimd ring) — 336KB instead of 672KB of HBM
    traffic, zero accuracy loss. The 22 numeric rows stay fp16 (176KB,
    HWDGE). M [104,256] fp16 on the second HWDGE ring.
  * Matmul orientation: M's D-half [104,128] is the STATIONARY operand;
    featT streams as the moving operand in 512-token slices (one PSUM
    bank per matmul, 16 matmuls/iter, 8192 streamed columns total).
    Output lands D-major: out[d, t].
  * Drains: PSUM [128,512] fp32 -> fp16 staging, alternating DVE/ACT.
  * Output: TWO 1MB DMAs ([128 partitions x 8KB contiguous] each — the
    max-bandwidth DMA shape) to DRAM out [256, 4096] fp16, one per
    D-half, on alternating HWDGE rings. Host transposes to [tok, D]
    while gathering shards (allowed unshard reshaping).

HBM traffic/iter: 0.57MB in + 2MB out = 2.6MB at the ~358 GB/s per-core
HBM limit -> ~7.2us floor; PE ~3.5us warm; drains ~3.5us split over two
engines. Numerical error vs fp32 reference: ~5e-4 max rel (fp16
features/weights/output, fp32 PSUM accumulation; the fp8 rows are exact).
"""

import os

import ml_dtypes
import numpy as np

import concourse.bass as bass
import concourse.mybir as mybir
import concourse.tile as tile
from concourse import bacc
from concourse.bass_utils import run_bass_kernel_spmd

# NTFF tracing is unavailable under axon (antenv.axon_hooks absent) —
# force it off so a stray BASS_TRACE=1 in the environment can't crash us.
os.environ["BASS_NEVER_TRACE"] = "1"

# Problem shape (hardcoded per harness contract)
B, S, D, MP, NA, NCARDS = 32, 1024, 256, 9, 8, 53
BS = B * S            # 32768 tokens
NCORES = 8
TOK = BS // NCORES    # 4096 tokens per core
KF = 104              # feature rows: 81 onehot + 1 bias + 21 numeric + 1 pad
K8 = 82               # fp8-storable rows (one-hot counts + bias; exact)
NBANK = 8             # PSUM banks per D-half
NTOKB = TOK // NBANK  # 512 tokens per bank-matmul

USE_FP8_IN = True

_CACHE = {}
LAST_RESULT = None    # BassKernelResults of the most recent run (for profiling)


def _build_program(reps=None, staggered=False, fp8_in=USE_FP8_IN):
    """Build + compile the per-core Bass program (identical on all cores).

    reps: if set, wrap the whole body in an on-device For_i loop that
    repeats the full workload (input DMA + matmuls + drains + output DMA)
    `reps` times — used only for timing (wall-clock slope over reps).
    """
    nc = bacc.Bacc("TRN2", target_bir_lowering=False, debug=False,
                   num_devices=NCORES)
    if fp8_in:
        f8_d = nc.dram_tensor("featT8", [K8, TOK], mybir.dt.float8e4,
                              kind="ExternalInput")
        f16_d = nc.dram_tensor("featT16", [KF - K8, TOK], mybir.dt.float16,
                               kind="ExternalInput")
    else:
        featT_d = nc.dram_tensor("featT", [KF, TOK], mybir.dt.float16,
                                 kind="ExternalInput")
    m_d = nc.dram_tensor("mcomb", [KF, D], mybir.dt.float16,
                         kind="ExternalInput")
    out_d = nc.dram_tensor("out", [D, TOK], mybir.dt.float16,
                           kind="ExternalOutput")

    with tile.TileContext(nc) as tc:
        with (
            tc.tile_pool(name="consts", bufs=2) as cpool,
            tc.tile_pool(name="psum", bufs=8, space="PSUM") as ppool,
            tc.tile_pool(name="outs", bufs=4) as opool,
        ):
            def body(_i=None):
                m_t = cpool.tile([KF, D], mybir.dt.float16, tag="mtile")
                nc.scalar.dma_start(m_t[:], m_d[:])
                f_t = cpool.tile([KF, TOK], mybir.dt.float16, tag="ftile")
                if fp8_in:
                    # SWDGE (gpsimd) casts fp8 -> fp16 at line rate; the
                    # one-hot/bias rows are small ints so the cast is exact.
                    nc.gpsimd.dma_start(f_t[0:K8, :], f8_d[:])
                    nc.sync.dma_start(f_t[K8:KF, :], f16_d[:])
                else:
                    h = TOK // 2
                    nc.sync.dma_start(f_t[:, 0:h], featT_d[:, 0:h])
                    nc.scalar.dma_start(f_t[:, h:TOK], featT_d[:, h:TOK])

                out_v = out_d[:].rearrange("(h p) t -> h p t", p=128)
                for dh in range(2):
                    stage = opool.tile([128, TOK], mybir.dt.float16)
                    for b in range(NBANK):
                        ps = ppool.tile([128, NTOKB], mybir.dt.float32)
                        nc.tensor.matmul(
                            ps[:],
                            m_t[:, dh * 128:(dh + 1) * 128],
                            f_t[:, b * NTOKB:(b + 1) * NTOKB],
                            start=True, stop=True)
                        sl = stage[:, b * NTOKB:(b + 1) * NTOKB]
                        if b % 2 == 0:
                            nc.vector.tensor_copy(sl, ps[:])
                        else:
                            nc.scalar.copy(sl, ps[:])
                    eng = nc.sync if dh == 0 else nc.scalar
                    eng.dma_start(out_v[dh], stage[:])

            if reps is None:
                body()
            else:
                with tc.For_i(0, reps, 1, staggered_reset=staggered):
                    body()

    nc.compile()
    return nc


def _fold_weights(card_table, hero_table, acting_table, nump_table,
                  scalar_W, scalar_b, blind_W, blind_b, bet_W, bet_b,
                  action_W, action_b, combine_W, combine_b):
    """Fold all tables/projections through combine_W into M [104, D] (fp32)."""
    W = np.asarray(combine_W, np.float32)          # [D, 8D]
    blk = [W[:, k * D:(k + 1) * D] for k in range(8)]
    # concat order: card, hero, acting, scalar, bet, action, nump, blind
    Wcard, Where, Wact, Wscal, Wbet, Waction, Wnump, Wblind = blk
    M = np.zeros((KF, D), np.float32)
    M[0:53] = np.asarray(card_table, np.float32) @ Wcard.T / 7.0
    M[53:62] = np.asarray(hero_table, np.float32) @ Where.T
    M[62:71] = np.asarray(acting_table, np.float32) @ Wact.T
    M[71:81] = np.asarray(nump_table, np.float32) @ Wnump.T
    M[81] = (np.asarray(combine_b, np.float32)
             + Wscal @ np.asarray(scalar_b, np.float32)
             + Wblind @ np.asarray(blind_b, np.float32)
             + Wbet @ np.asarray(bet_b, np.float32)
             + Waction @ np.asarray(action_b, np.float32))
    M[82:84] = (Wscal @ np.asarray(scalar_W, np.float32)).T
    M[84:86] = (Wblind @ np.asarray(blind_W, np.float32)).T
    M[86:95] = (Wbet @ np.asarray(bet_W, np.float32)).T
    M[95:103] = (Waction @ np.asarray(action_W, np.float32)).T
    return M


def _build_features(cards, hero_pos, acting_pos, num_players,
                    scalars, blinds, bets, action, mask):
    """Build featT [104, BS] fp32 (mask folded in)."""
    cards = np.asarray(cards).reshape(BS, 7).astype(np.int64)
    hero = np.asarray(hero_pos).reshape(BS).astype(np.int64)
    act = np.asarray(acting_pos).reshape(BS).astype(np.int64)
    nump = np.asarray(num_players).reshape(BS).astype(np.int64)
    msk = np.asarray(mask, np.float32).reshape(BS)

    feat = np.zeros((BS, KF), np.float32)
    ar53 = np.arange(NCARDS, dtype=np.int64)
    feat[:, 0:53] = (cards[:, :, None] == ar53).sum(axis=1, dtype=np.float32)
    feat[:, 53:62] = hero[:, None] == np.arange(9)
    feat[:, 62:71] = act[:, None] == np.arange(9)
    feat[:, 71:81] = nump[:, None] == np.arange(10)
    feat[:, 81] = 1.0
    feat[:, 0:82] *= msk[:, None]
    num = np.concatenate([
        np.asarray(scalars, np.float32).reshape(BS, 2),
        np.asarray(blinds, np.float32).reshape(BS, 2),
        np.asarray(bets, np.float32).reshape(BS, MP),
        np.asarray(action, np.float32).reshape(BS, NA),
    ], axis=1) * msk[:, None]
    feat[:, 82:103] = num          # num already carries the mask
    return feat.T


def _prepare_in_maps(inputs):
    """inputs: full dict keyed as setup_inputs(). Returns per-core in_maps."""
    wkeys = ["card_table", "hero_table", "acting_table", "nump_table",
             "scalar_W", "scalar_b", "blind_W", "blind_b", "bet_W", "bet_b",
             "action_W", "action_b", "combine_W", "combine_b"]
    fkeys = ["cards", "hero_pos", "acting_pos", "num_players",
             "scalars", "blinds", "bets", "action", "mask"]
    M = _fold_weights(**{k: inputs[k] for k in wkeys})
    featT = _build_features(*[inputs[k] for k in fkeys])
    m16 = np.ascontiguousarray(M, dtype=np.float16)
    in_maps = []
    for i in range(NCORES):
        blk = featT[:, i * TOK:(i + 1) * TOK]
        if USE_FP8_IN:
            in_maps.append({
                "featT8": np.ascontiguousarray(blk[0:K8]).astype(
                    ml_dtypes.float8_e4m3),
                "featT16": np.ascontiguousarray(blk[K8:KF], dtype=np.float16),
                "mcomb": m16,
            })
        else:
            in_maps.append({
                "featT": np.ascontiguousarray(blk, dtype=np.float16),
                "mcomb": m16,
            })
    return in_maps


def kernel(cards, hero_pos, acting_pos, num_players, scalars, blinds, bets,
           action, mask, card_table, hero_table, acting_table, nump_table,
           scalar_W, scalar_b, blind_W, blind_b, bet_W, bet_b,
           action_W, action_b, combine_W, combine_b):
    global LAST_RESULT
    if "nc" not in _CACHE:
        _CACHE["nc"] = _build_program()
    nc = _CACHE["nc"]

    in_maps = _prepare_in_maps(dict(
        cards=cards, hero_pos=hero_pos, acting_pos=acting_pos,
        num_players=num_players, scalars=scalars, blinds=blinds, bets=bets,
        action=action, mask=mask, card_table=card_table,
        hero_table=hero_table, acting_table=acting_table,
        nump_table=nump_table, scalar_W=scalar_W, scalar_b=scalar_b,
        blind_W=blind_W, blind_b=blind_b, bet_W=bet_W, bet_b=bet_b,
        action_W=action_W, action_b=action_b, combine_W=combine_W,
        combine_b=combine_b))

    res = run_bass_kernel_spmd(nc, in_maps, core_ids=list(range(NCORES)))
    LAST_RESULT = res
    # device output is D-major [D, TOK] per core; transpose while gathering
    out = np.concatenate(
        [np.asarray(res.results[i]["out"], np.float32).T
         for i in range(NCORES)], axis=0)
    return out.reshape(B, S, D)


# revision 5
# speedup vs baseline: 1.4201x; 1.4201x over previous
"""Trainium2 Bass kernel for nn_EventSequenceEmbedder.

Strategy
--------
The whole module is algebraically folded on the host into a single small
matrix product per token:

    out[t, :] = featT[:, t] . M  (masked)

where
  * M [104, 256] is built once from the weights: each embedding table and
    each linear projection is folded through its combine_W column block
    (pure weight preprocessing), all biases collapse into one bias row.
  * featT [104, BS] is the per-token sparse feature vector (row order):
      rows 0:53    card multihot (counts of the 7 card ids; /7 folded in M)
      rows 53:62   hero one-hot
      rows 62:71   acting one-hot
      rows 71:81   num_players one-hot
      row  81      ones (bias row)
      rows 82:103  raw numeric features (scalars2, blinds2, bets9, action8)
      row  103     zero padding
    The whole featT is scaled by mask, which reproduces `out * mask` exactly.

Sharding: data-parallel over tokens, 8 contiguous blocks of 4096.

Per-core device program (v3 — chunk-pipelined, dedicated DMA rings):
  * Matmul orientation: M's D-half [104,128] is the STATIONARY operand;
    featT streams as the moving operand in 512-token slices (one PSUM
    bank per matmul, 16 matmuls/iter, 8192 streamed columns total).
    Output lands D-major: out[d, t], so every output DMA is a
    [128 partitions x contiguous] max-bandwidth shape and the host
    transposes to [tok, D] while gathering shards.
  * Token dim is cut into 4 chunks of 1024. Chunk c's input DMA, the
    4 matmuls (2 D-halves x 2 PSUM banks), the PSUM drains and the 2
    output DMAs form an independent pipeline stage, so input stream,
    PE, drain engines and output stream all run concurrently.
  * Ring discipline: ALL input DMAs ride the sync(SP) HWDGE ring, ALL
    output DMAs ride the scalar(ACT) HWDGE ring (a single ring still
    spreads each DMA across all 16 SDMA engines, so one ring sustains
    full bandwidth) — an output DMA waiting on drains can never block
    an input DMA behind it in ring-FIFO order.
  * Optional fp8 input (USE_FP8_IN): rows 0:82 (one-hot counts + bias:
    small integers, EXACTLY representable in fp8e4) are stored in DRAM
    as fp8 and cast to fp16 during SWDGE DMAs on the gpsimd ring (a
    third, independent issue path) — 336KB instead of 672KB of HBM
    traffic, zero accuracy loss. The 22 numeric rows stay fp16 (HWDGE).
  * Drains: PSUM [128,512] fp32 -> fp16 staging, alternating DVE/ACT.

HBM traffic/iter: 0.57-0.9MB in + 2MB out at the ~358 GB/s per-core
HBM limit -> ~7.2-8.1us floor; PE ~3.5us warm; drains ~3.5us split over
two engines. Numerical error vs fp32 reference: ~5e-4 max rel (fp16
features/weights/output, fp32 PSUM accumulation; the fp8 rows are exact).
"""

import os

import ml_dtypes
import numpy as np

import concourse.bass as bass
import concourse.mybir as mybir
import concourse.tile as tile
from concourse import bacc
from concourse.bass_utils import run_bass_kernel_spmd

# NTFF tracing is unavailable under axon (antenv.axon_hooks absent) —
# force it off so a stray BASS_TRACE=1 in the environment can't crash us.
os.environ["BASS_NEVER_TRACE"] = "1"

# Problem shape (hardcoded per harness contract)
B, S, D, MP, NA, NCARDS = 32, 1024, 256, 9, 8, 53
BS = B * S            # 32768 tokens
NCORES = 8
TOK = BS // NCORES    # 4096 tokens per core
KF = 104              # feature rows: 81 onehot + 1 bias + 21 numeric + 1 pad
K8 = 82               # fp8-storable rows (one-hot counts + bias; exact)
NBANK = 8             # PSUM banks per D-half
NTOKB = TOK // NBANK  # 512 tokens per bank-matmul

USE_FP8_IN = False
NCHUNK = 4            # token chunks per iteration (pipeline granularity)
CTOK = TOK // NCHUNK  # 1024 tokens per chunk

_CACHE = {}
LAST_RESULT = None    # BassKernelResults of the most recent run (for profiling)


def _build_program(reps=None, staggered=False, fp8_in=USE_FP8_IN):
    """Build + compile the per-core Bass program (identical on all cores).

    reps: if set, wrap the whole body in an on-device For_i loop that
    repeats the full workload (input DMA + matmuls + drains + output DMA)
    `reps` times — used only for timing (wall-clock slope over reps).
    """
    nc = bacc.Bacc("TRN2", target_bir_lowering=False, debug=False,
                   num_devices=NCORES)
    if fp8_in:
        f8_d = nc.dram_tensor("featT8", [K8, TOK], mybir.dt.float8e4,
                              kind="ExternalInput")
        f16_d = nc.dram_tensor("featT16", [KF - K8, TOK], mybir.dt.float16,
                               kind="ExternalInput")
    else:
        featT_d = nc.dram_tensor("featT", [KF, TOK], mybir.dt.float16,
                                 kind="ExternalInput")
    m_d = nc.dram_tensor("mcomb", [KF, D], mybir.dt.float16,
                         kind="ExternalInput")
    out_d = nc.dram_tensor("out", [D, TOK], mybir.dt.float16,
                           kind="ExternalOutput")

    with tile.TileContext(nc) as tc:
        with (
            tc.tile_pool(name="consts", bufs=2) as cpool,
            tc.tile_pool(name="psum", bufs=8, space="PSUM") as ppool,
            tc.tile_pool(name="outs", bufs=4) as opool,
        ):
            def body(_i=None):
                m_t = cpool.tile([KF, D], mybir.dt.float16, tag="mtile")
                nc.sync.dma_start(m_t[:], m_d[:])
                f_t = cpool.tile([KF, TOK], mybir.dt.float16, tag="ftile")
                for c in range(NCHUNK):
                    cs = slice(c * CTOK, (c + 1) * CTOK)
                    if fp8_in:
                        # SWDGE (gpsimd) casts fp8 -> fp16 in-flight; the
                        # one-hot/bias rows are small ints, cast is exact.
                        nc.gpsimd.dma_start(f_t[0:K8, cs], f8_d[:, cs])
                        nc.sync.dma_start(f_t[K8:KF, cs], f16_d[:, cs])
                    else:
                        nc.sync.dma_start(f_t[:, cs], featT_d[:, cs])

                out_v = out_d[:].rearrange("(h p) t -> h p t", p=128)
                bpc = CTOK // NTOKB   # PSUM banks per chunk per D-half
                for c in range(NCHUNK):
                    for dh in range(2):
                        stage = opool.tile([128, CTOK], mybir.dt.float16)
                        for b in range(bpc):
                            t0 = c * CTOK + b * NTOKB
                            ps = ppool.tile([128, NTOKB], mybir.dt.float32)
                            nc.tensor.matmul(
                                ps[:],
                                m_t[:, dh * 128:(dh + 1) * 128],
                                f_t[:, t0:t0 + NTOKB],
                                start=True, stop=True)
                            sl = stage[:, b * NTOKB:(b + 1) * NTOKB]
                            if (2 * dh + b) % 2 == 0:
                                nc.vector.tensor_copy(sl, ps[:])
                            else:
                                nc.scalar.copy(sl, ps[:])
                        nc.scalar.dma_start(
                            out_v[dh][:, c * CTOK:(c + 1) * CTOK], stage[:])

            if reps is None:
                body()
            else:
                with tc.For_i(0, reps, 1, staggered_reset=staggered):
                    body()

    nc.compile()
    return nc


def _fold_weights(card_table, hero_table, acting_table, nump_table,
                  scalar_W, scalar_b, blind_W, blind_b, bet_W, bet_b,
                  action_W, action_b, combine_W, combine_b):
    """Fold all tables/projections through combine_W into M [104, D] (fp32)."""
    W = np.asarray(combine_W, np.float32)          # [D, 8D]
    blk = [W[:, k * D:(k + 1) * D] for k in range(8)]
    # concat order: card, hero, acting, scalar, bet, action, nump, blind
    Wcard, Where, Wact, Wscal, Wbet, Waction, Wnump, Wblind = blk
    M = np.zeros((KF, D), np.float32)
    M[0:53] = np.asarray(card_table, np.float32) @ Wcard.T / 7.0
    M[53:62] = np.asarray(hero_table, np.float32) @ Where.T
    M[62:71] = np.asarray(acting_table, np.float32) @ Wact.T
    M[71:81] = np.asarray(nump_table, np.float32) @ Wnump.T
    M[81] = (np.asarray(combine_b, np.float32)
             + Wscal @ np.asarray(scalar_b, np.float32)
             + Wblind @ np.asarray(blind_b, np.float32)
             + Wbet @ np.asarray(bet_b, np.float32)
             + Waction @ np.asarray(action_b, np.float32))
    M[82:84] = (Wscal @ np.asarray(scalar_W, np.float32)).T
    M[84:86] = (Wblind @ np.asarray(blind_W, np.float32)).T
    M[86:95] = (Wbet @ np.asarray(bet_W, np.float32)).T
    M[95:103] = (Waction @ np.asarray(action_W, np.float32)).T
    return M


def _build_features(cards, hero_pos, acting_pos, num_players,
                    scalars, blinds, bets, action, mask):
    """Build featT [104, BS] fp32 (mask folded in)."""
    cards = np.asarray(cards).reshape(BS, 7).astype(np.int64)
    hero = np.asarray(hero_pos).reshape(BS).astype(np.int64)
    act = np.asarray(acting_pos).reshape(BS).astype(np.int64)
    nump = np.asarray(num_players).reshape(BS).astype(np.int64)
    msk = np.asarray(mask, np.float32).reshape(BS)

    feat = np.zeros((BS, KF), np.float32)
    ar53 = np.arange(NCARDS, dtype=np.int64)
    feat[:, 0:53] = (cards[:, :, None] == ar53).sum(axis=1, dtype=np.float32)
    feat[:, 53:62] = hero[:, None] == np.arange(9)
    feat[:, 62:71] = act[:, None] == np.arange(9)
    feat[:, 71:81] = nump[:, None] == np.arange(10)
    feat[:, 81] = 1.0
    feat[:, 0:82] *= msk[:, None]
    num = np.concatenate([
        np.asarray(scalars, np.float32).reshape(BS, 2),
        np.asarray(blinds, np.float32).reshape(BS, 2),
        np.asarray(bets, np.float32).reshape(BS, MP),
        np.asarray(action, np.float32).reshape(BS, NA),
    ], axis=1) * msk[:, None]
    feat[:, 82:103] = num          # num already carries the mask
    return feat.T


def _prepare_in_maps(inputs):
    """inputs: full dict keyed as setup_inputs(). Returns per-core in_maps."""
    wkeys = ["card_table", "hero_table", "acting_table", "nump_table",
             "scalar_W", "scalar_b", "blind_W", "blind_b", "bet_W", "bet_b",
             "action_W", "action_b", "combine_W", "combine_b"]
    fkeys = ["cards", "hero_pos", "acting_pos", "num_players",
             "scalars", "blinds", "bets", "action", "mask"]
    M = _fold_weights(**{k: inputs[k] for k in wkeys})
    featT = _build_features(*[inputs[k] for k in fkeys])
    m16 = np.ascontiguousarray(M, dtype=np.float16)
    in_maps = []
    for i in range(NCORES):
        blk = featT[:, i * TOK:(i + 1) * TOK]
        if USE_FP8_IN:
            in_maps.append({
                "featT8": np.ascontiguousarray(blk[0:K8]).astype(
                    ml_dtypes.float8_e4m3),
                "featT16": np.ascontiguousarray(blk[K8:KF], dtype=np.float16),
                "mcomb": m16,
            })
        else:
            in_maps.append({
                "featT": np.ascontiguousarray(blk, dtype=np.float16),
                "mcomb": m16,
            })
    return in_maps


def kernel(cards, hero_pos, acting_pos, num_players, scalars, blinds, bets,
           action, mask, card_table, hero_table, acting_table, nump_table,
           scalar_W, scalar_b, blind_W, blind_b, bet_W, bet_b,
           action_W, action_b, combine_W, combine_b):
    global LAST_RESULT
    if "nc" not in _CACHE:
        _CACHE["nc"] = _build_program()
    nc = _CACHE["nc"]

    in_maps = _prepare_in_maps(dict(
        cards=cards, hero_pos=hero_pos, acting_pos=acting_pos,
        num_players=num_players, scalars=scalars, blinds=blinds, bets=bets,
        action=action, mask=mask, card_table=card_table,
        hero_table=hero_table, acting_table=acting_table,
        nump_table=nump_table, scalar_W=scalar_W, scalar_b=scalar_b,
        blind_W=blind_W, blind_b=blind_b, bet_W=bet_W, bet_b=bet_b,
        action_W=action_W, action_b=action_b, combine_W=combine_W,
        combine_b=combine_b))

    res = run_bass_kernel_spmd(nc, in_maps, core_ids=list(range(NCORES)))
    LAST_RESULT = res
    # device output is D-major [D, TOK] per core; transpose while gathering
    out = np.concatenate(
        [np.asarray(res.results[i]["out"], np.float32).T
         for i in range(NCORES)], axis=0)
    return out.reshape(B, S, D)
